# revision 1
# baseline (speedup 1.0000x reference)
"""Self-attention (IntraSelfAttention) kernel for Trainium2, 8-core data parallel.

Math (per batch element b, on one core):
    E   = exp(A @ A.T)                  # [S,S], symmetric, NO masking needed
    U   = E @ (A * m[:,None])           # fold col-mask into V operand
    r   = E @ m                         # rowsum of col-masked E (extra column)
    out = U / (r + eps)                 # row-mask applied on host (x*m_s)

Symmetry of E means the tiles produced by the QK matmul ([t_part, s_free])
are directly usable as lhsT for the AV matmul — no transposes anywhere.

Hardware constraint shaping the code: most engine instruction structs accept
only ONE sync-wait, so the dep graph is arranged such that no instruction
ever needs two new semaphore waits (5 total DMA instructions = no HWDGE
queue reuse; hand-double-buffered PSUM tiles in the AV phase so slot reuse
is same-tile WAW, which is same-engine ordered and free).
"""

import os
import numpy as np

try:
    import concourse.bass as bass
except ImportError:
    import sys

    sys.path.insert(0, "/opt/trn_rl_repo")
    import concourse.bass as bass

import concourse.mybir as mybir
import concourse.tile as tile
from concourse import bass_utils
from concourse.tile_sem_assignment import PROC_NAME_TO_IDX

_IDX2PROC = {v: k for k, v in PROC_NAME_TO_IDX.items()}


def _split_drain_and_barrier(self, tick_clock, wait_clock):
    """Replacement for TileContext._drain_and_barrier.

    The stock version attaches every outstanding semaphore wait to the single
    kernel-tail Drain instruction; walrus's per-instruction sync-wait capacity
    is tiny, so with >4-ish sems the NEFF fails codegen ("Too many sync wait
    commands"). Split the waits across single-wait sequencer nops instead.
    """
    nc = self.nc
    gc = tick_clock.global_clock
    ticks = list(gc)
    for idx, sem in self.sems.allocated().items():
        tick = ticks[idx]
        if tick <= 0:
            continue
        name = _IDX2PROC.get(idx, "")
        val = tick * (16 if name.startswith("DMA") else 1)
        nc.sync.nop().wait_op(sem, val, "sem-ge")
    nc.sync.drain()
    nc.all_engine_barrier()
    popped = nc._tile_sem_poison_stack.pop()
    assert popped is self._sem_poison
    nc.clear_and_free_semaphores(list(self.sems.allocated().values()))
    nc.all_engine_barrier()


tile.TileContext._drain_and_barrier = _split_drain_and_barrier

B, S, D = 8, 1024, 768
NCORES = 8
EPS = 1e-7
P = 128
KT = D // P  # 6  k-tiles for QK (contract over D)
NT = S // P  # 8  128-blocks of S
NJ = S // 512  # 2  512-wide column groups of S
DV = D + 1  # 769 = [A*m | m]
NG = 2  # output DMA groups (4 row-blocks each)

MM_DT = mybir.dt.float16
NP_DT = np.float16
F32 = mybir.dt.float32

_cache = {}


def _build():
    nc = bass.Bass()
    at = nc.declare_dram_parameter("at", [D, S], MM_DT, isOutput=False)
    av = nc.declare_dram_parameter("av", [S, DV], MM_DT, isOutput=False)
    out = nc.declare_dram_parameter("out", [S, D], F32, isOutput=True)

    with tile.TileContext(nc) as tc:
        with (
            tc.tile_pool(name="w", bufs=1) as wpool,
            tc.tile_pool(name="e", bufs=1) as epool,
            tc.tile_pool(name="qkps", bufs=4, space="PSUM") as qkps,
            tc.tile_pool(name="avps", bufs=1, space="PSUM") as avps,
            tc.tile_pool(name="o", bufs=1) as opool,
            tc.tile_pool(name="s", bufs=8) as spool,
        ):
            # --- inputs: 3 DMA instructions, one HWDGE queue each ---
            atv = at.rearrange("(u k p) s -> u p k s", u=2, p=P)  # [2, P, 3, S]
            att = []
            for u in range(2):
                t_ = wpool.tile([P, KT // 2, S], MM_DT, name=f"att{u}", tag=f"att{u}")
                nc.sync.dma_start(t_[:, :, :], atv[u])
                att.append(t_)
            avt = wpool.tile([P, NT, DV], MM_DT)
            nc.sync.dma_start(avt[:, :, :], av.rearrange("(t p) d -> p t d", p=P))

            def at_k(k):
                return att[k // 3][:, k % 3, :]

            E = [
                epool.tile([P, S], MM_DT, name=f"E{t}", tag=f"E{t}")
                for t in range(NT)
            ]  # E[t*P+p, s]

            # --- QK^T + exp:  E[i*P+p, s] for s-column group j ---
            for j in range(NJ):
                for h in range(2):  # half-columns of 4 psum banks
                    ii = list(range(4 * h, 4 * h + 4))
                    ps = {
                        i: qkps.tile([P, 512], F32, tag="qk", name=f"qk_{j}_{i}")
                        for i in ii
                    }
                    for k in range(KT):
                        for i in ii:
                            nc.tensor.matmul(
                                ps[i][:, :],
                                at_k(k)[:, i * P : (i + 1) * P],
                                at_k(k)[:, j * 512 : (j + 1) * 512],
                                start=(k == 0),
                                stop=(k == KT - 1),
                            )
                    for i in ii:
                        nc.scalar.activation(
                            E[i][:, j * 512 : (j + 1) * 512],
                            ps[i][:, :],
                            mybir.ActivationFunctionType.Exp,
                        )

            # --- AV: U_ext = E @ [A*m | m], then scale rows by 1/(r+eps) ---
            ots = [
                opool.tile([P, NT // NG, D], F32, name=f"ot{g}", tag=f"ot{g}")
                for g in range(NG)
            ]
            pabuf = [
                avps.tile([P, 512], F32, tag=f"pa{x}", name=f"pa{x}") for x in range(2)
            ]
            pbbuf = [
                avps.tile([P, 257], F32, tag=f"pb{x}", name=f"pb{x}") for x in range(2)
            ]
            MPG = NT // NG  # m-blocks per output DMA group
            for m in range(NT):
                pa = pabuf[m % 2]
                pb = pbbuf[m % 2]
                for t in range(NT):
                    lhsT = E[t][:, m * P : (m + 1) * P]
                    nc.tensor.matmul(
                        pa[:, :],
                        lhsT,
                        avt[:, t, 0:512],
                        start=(t == 0),
                        stop=(t == NT - 1),
                    )
                    nc.tensor.matmul(
                        pb[:, :],
                        lhsT,
                        avt[:, t, 512:DV],
                        start=(t == 0),
                        stop=(t == NT - 1),
                    )
                rtmp = spool.tile([P, 1], F32, tag=f"rtmp{m}", name=f"rtmp{m}")
                nc.vector.tensor_scalar_add(rtmp[:, :], pb[:, 256:257], EPS)
                rinv = spool.tile([P, 1], F32, tag=f"rinv{m}", name=f"rinv{m}")
                nc.vector.reciprocal(rinv[:, :], rtmp[:, :])
                ot = ots[m // MPG][:, m % MPG, :]
                nc.vector.tensor_scalar_mul(ot[:, 0:512], pa[:, :], rinv[:, :])
                nc.vector.tensor_scalar_mul(ot[:, 512:D], pb[:, 0:256], rinv[:, :])
                if m % MPG == MPG - 1:
                    g = m // MPG
                    nc.sync.dma_start(
                        out[g * MPG * P : (g + 1) * MPG * P, :].rearrange(
                            "(b p) d -> p b d", p=P
                        ),
                        ots[g][:, :, :],
                    )

    return nc


def _get_nc():
    if "nc" not in _cache:
        _cache["nc"] = _build()
    return _cache["nc"]


def kernel(input_a, input_mask, _trace=False, **_kw):
    A = np.asarray(input_a, dtype=np.float32)  # [B, S, D]
    M = np.asarray(input_mask)  # [B, S] int32

    in_maps = []
    mfs = []
    for b in range(B):
        a = A[b]
        mf = M[b].astype(np.float32)
        mfs.append(mf)
        at = np.ascontiguousarray(a.T).astype(NP_DT)
        av = np.empty((S, DV), NP_DT)
        av[:, :D] = (a * mf[:, None]).astype(NP_DT)
        av[:, D] = mf.astype(NP_DT)
        in_maps.append({"at": at, "av": av})

    nc = _get_nc()
    res = bass_utils.run_bass_kernel_spmd(
        nc, in_maps, core_ids=list(range(NCORES)), trace=_trace
    )
    outp = np.stack([res.results[b]["out"] for b in range(B)], axis=0)
    outp *= np.stack(mfs, axis=0)[:, :, None]  # row mask
    if _trace:
        kernel.last_results = res
    return outp.astype(np.float32)

